# revision 5
# baseline (speedup 1.0000x reference)
"""CapsuleLayer routing kernel for Trainium2, 8 NeuronCores.

Math (per reference):
    u[b,i,n,d] = sum_k W[0,i,n,d,k] * x[b,i,k]
    s[b,i,n]   = sum_d u[b,i,n,d]
    b1 = s/32 ; c1 = softmax_n(b1) ; b2 = b1 + c1*s ; c2 = softmax_n(b2)
    out[b,n,d] = squash(sum_i c2[b,i,n] * u[b,i,n,d])

(The per-iteration `outputs` of routing iters 0..1 are dead code; routing is
fully local per (b,i); only the final i-sum couples cores.)

Sharding: tensor-parallel over IN_CAPS (2048/8 = 256 i per core). Each core
returns its partial pre-squash sum [32, 512]; the host adds the 8 partials
(the unshard for a reduction-sharded output) and applies squash.

Device layout (host-prepared, per core):
  wt  [32, 128, 512] f32: wt[m, 64*h + 16*j + k, 16*n + d] = W[0, i0+4*(2m+h)+j, n, d, k]
      -> DMA tile [128p, 512f], 2KB contiguous per partition.
  xbd [32, 128, 128] f32: block-diagonal stationary tiles;
      xbd[m, 64*h + 16*j + c_j, 32*j + b] = x[b, i0+4*(2m+h)+j, k]
  matmul per group g=2m+h: lhsT = xbd[64h:64h+64, 128m:...] (K=64 = 4i*16k,
      M=128 = 4i*32b), rhs = wt[64h:64h+64, :] (N=512 = 32n*16d)
      -> u_g[32j+b, 16n+d] in PSUM.
"""

import os
import sys

import numpy as np

sys.path.insert(0, "/opt/trn_rl_repo")

import concourse.bass as bass
import concourse.bacc as bacc
import concourse.mybir as mybir
import concourse.tile as tile
from concourse.bass_utils import run_bass_kernel_spmd

B = 32
IN_CAPS = 2048
IN_DIM = 16
NUM_CAPS = 32
DIM_CAPS = 16
NCORES = 8
NI = IN_CAPS // NCORES  # 256
G = NI // 4  # 64 groups of 4 i's
M = G // 2  # 32 pair-tiles
ND = NUM_CAPS * DIM_CAPS  # 512
EPS = 1e-7

F32 = mybir.dt.float32
BF16 = mybir.dt.bfloat16
X = mybir.AxisListType.X
ADD = mybir.AluOpType.add
MULT = mybir.AluOpType.mult

LAST_RESULTS = None
_NC = None


def _kernel_body(tc):
    nc = tc.nc
    xbd_d = nc.dram_tensor("xbd", [M, 128, 128], F32, kind="ExternalInput").ap()
    wt_d = nc.dram_tensor("wt", [M, 128, ND], F32, kind="ExternalInput").ap()
    out_d = nc.dram_tensor("partial", [B, ND], F32, kind="ExternalOutput").ap()

    from contextlib import ExitStack

    with ExitStack() as ctx:
        const_pool = ctx.enter_context(tc.tile_pool(name="const", bufs=1))
        wpool = ctx.enter_context(tc.tile_pool(name="w", bufs=M))
        psum = ctx.enter_context(tc.tile_pool(name="ps", bufs=4, space="PSUM"))
        big = ctx.enter_context(tc.tile_pool(name="big", bufs=1))
        small = ctx.enter_context(tc.tile_pool(name="small", bufs=1))

        # Stationary block-diag x tiles: [128, M*128] bf16 (cast during DMA).
        xbd_sb = const_pool.tile([128, M * 128], BF16)
        nc.gpsimd.dma_start(
            out=xbd_sb[:].rearrange("p (m c) -> p m c", c=128),
            in_=xbd_d.rearrange("m p c -> p m c"),
        )

        U = big.tile([128, G * ND], BF16)  # evicted u, 64KB/partition
        S = small.tile([128, G * NUM_CAPS], F32)  # s[b,i,n]

        for m in range(M):
            wtile = wpool.tile([128, ND], BF16)
            nc.gpsimd.dma_start(out=wtile[:], in_=wt_d[m])  # f32 -> bf16 cast
            for h in range(2):
                g = 2 * m + h
                u_ps = psum.tile([128, ND], F32)
                nc.tensor.matmul(
                    u_ps[:],
                    lhsT=xbd_sb[64 * h : 64 * (h + 1), m * 128 : (m + 1) * 128],
                    rhs=wtile[64 * h : 64 * (h + 1), :],
                    start=True,
                    stop=True,
                )
                # Evict PSUM -> SBUF bf16; alternate engines to split the load.
                dst = U[:, g * ND : (g + 1) * ND]
                if g % 2 == 0:
                    nc.scalar.copy(dst, u_ps[:])
                else:
                    nc.vector.tensor_copy(dst, u_ps[:])
            if m % 4 == 3:
                blk = m // 4  # 8 groups per reduce batch
                nc.vector.tensor_reduce(
                    out=S[:, blk * 8 * NUM_CAPS : (blk + 1) * 8 * NUM_CAPS],
                    in_=U[:, blk * 8 * ND : (blk + 1) * 8 * ND].rearrange(
                        "p (q d) -> p q d", d=DIM_CAPS
                    ),
                    axis=X,
                    op=ADD,
                )

        # Routing (batched, fully local per (partition=(j,b), group)).
        E1 = small.tile([128, G * NUM_CAPS], BF16)
        nc.scalar.activation(
            E1[:], S[:], mybir.ActivationFunctionType.Exp, scale=1.0 / 32.0
        )
        Z1 = small.tile([128, G], F32)
        nc.vector.tensor_reduce(
            out=Z1[:],
            in_=E1[:].rearrange("p (g n) -> p g n", n=NUM_CAPS),
            axis=X,
            op=ADD,
        )
        R1 = small.tile([128, G], F32)
        nc.vector.reciprocal(R1[:], Z1[:])
        c1 = small.tile([128, G * NUM_CAPS], BF16)
        nc.vector.tensor_mul(
            c1[:].rearrange("p (g n) -> p g n", n=NUM_CAPS),
            E1[:].rearrange("p (g n) -> p g n", n=NUM_CAPS),
            R1[:].unsqueeze(2).broadcast_to((128, G, NUM_CAPS)),
        )
        B2 = small.tile([128, G * NUM_CAPS], F32)
        nc.vector.scalar_tensor_tensor(
            out=B2[:], in0=c1[:], scalar=1.0 / 32.0, in1=S[:], op0=ADD, op1=MULT
        )
        E2 = small.tile([128, G * NUM_CAPS], BF16)
        nc.scalar.activation(E2[:], B2[:], mybir.ActivationFunctionType.Exp)
        Z2 = small.tile([128, G], F32)
        nc.vector.tensor_reduce(
            out=Z2[:],
            in_=E2[:].rearrange("p (g n) -> p g n", n=NUM_CAPS),
            axis=X,
            op=ADD,
        )
        R2 = small.tile([128, G], F32)
        nc.vector.reciprocal(R2[:], Z2[:])
        c2 = small.tile([128, G * NUM_CAPS], BF16)
        nc.vector.tensor_mul(
            c2[:].rearrange("p (g n) -> p g n", n=NUM_CAPS),
            E2[:].rearrange("p (g n) -> p g n", n=NUM_CAPS),
            R2[:].unsqueeze(2).broadcast_to((128, G, NUM_CAPS)),
        )

        # Phase 3: U *= c2 (broadcast over d), then tree-reduce over g.
        nc.vector.tensor_mul(
            U[:].rearrange("p (g n d) -> p g n d", n=NUM_CAPS, d=DIM_CAPS),
            U[:].rearrange("p (g n d) -> p g n d", n=NUM_CAPS, d=DIM_CAPS),
            c2[:]
            .rearrange("p (g n) -> p g n", n=NUM_CAPS)
            .unsqueeze(3)
            .broadcast_to((128, G, NUM_CAPS, DIM_CAPS)),
        )
        w = G
        while w > 1:
            hw_ = w // 2
            nc.vector.tensor_add(
                U[:, : hw_ * ND], U[:, : hw_ * ND], U[:, hw_ * ND : w * ND]
            )
            w = hw_

        # Fold the 4 j-blocks of partitions: partial[b] = sum_j ACC[32j+b].
        tshift = small.tile([32, 3 * ND], BF16)
        for q in range(3):
            nc.sync.dma_start(
                out=tshift[:, q * ND : (q + 1) * ND],
                in_=U[32 * (q + 1) : 32 * (q + 2), :ND],
            )
        P0 = small.tile([32, ND], F32)
        nc.vector.tensor_add(P0[:], U[0:32, :ND], tshift[:, 0:ND])
        nc.vector.tensor_add(P0[:], P0[:], tshift[:, ND : 2 * ND])
        nc.vector.tensor_add(P0[:], P0[:], tshift[:, 2 * ND : 3 * ND])

        nc.sync.dma_start(out=out_d[:], in_=P0[:])


def build_nc():
    nc = bacc.Bacc(
        "TRN2",
        target_bir_lowering=False,
        debug=False,
        enable_asserts=False,
        num_devices=NCORES,
    )
    with tile.TileContext(nc) as tc:
        _kernel_body(tc)
    nc.compile()
    return nc


def _get_nc():
    global _NC
    if _NC is None:
        _NC = build_nc()
    return _NC


def prep_core_inputs(x, W, c):
    """Host-side shard + layout prep for core c (pure relayout, no math)."""
    sl = slice(c * NI, (c + 1) * NI)
    xt = np.ascontiguousarray(
        np.transpose(x[:, sl, :], (1, 2, 0))
    )  # [NI, k, b]
    xt = xt.reshape(G, 4, IN_DIM, B)
    xbd = np.zeros((G, 64, 128), np.float32)
    for j in range(4):
        xbd[:, 16 * j : 16 * (j + 1), 32 * j : 32 * (j + 1)] = xt[:, j]
    xbd = np.ascontiguousarray(xbd.reshape(M, 128, 128))
    ws = W[0, sl]  # [NI, n, d, k]
    wt = np.ascontiguousarray(np.transpose(ws, (0, 3, 1, 2))).reshape(G, 64, ND)
    wt = np.ascontiguousarray(wt.reshape(M, 128, ND))
    return {"xbd": xbd, "wt": wt}


def kernel(x, W):
    global LAST_RESULTS
    x = np.asarray(x, dtype=np.float32)
    W = np.asarray(W, dtype=np.float32)
    in_maps = [prep_core_inputs(x, W, c) for c in range(NCORES)]
    res = run_bass_kernel_spmd(_get_nc(), in_maps, core_ids=list(range(NCORES)))
    LAST_RESULTS = res
    v = np.sum(
        [r["partial"] for r in res.results], axis=0, dtype=np.float32
    ).reshape(B, NUM_CAPS, DIM_CAPS)
    z = np.sum(v * v, axis=-1, keepdims=True, dtype=np.float32)
    scale = z / (1.0 + z) / np.sqrt(z + EPS)
    return (scale * v).astype(np.float32)
